# revision 23
# baseline (speedup 1.0000x reference)
"""Masked dot-product attention (B=64, S=1024, D=64) on 8 Trainium2 NeuronCores.

Strategy (per core, 8 batches, valid-length-specialized to n k-chunks/batch):
  - Two fused input DMAs per batch: head tile [Qhalf | bias | Kt chunk0]
    (1.3KB/partition, unblocks QK+exp immediately) and bulk tile
    [Kt chunks 1.. | V'] that only gates later chunks and PV.
  - S^T chunks [k=128, q=1024] = K_chunk @ Q^T on PE, D=64 contraction on
    partitions; the two 64-row strips of the PE array compute the two q-halves
    of the SAME chunk concurrently (tile_position row packing, no Q dup).
  - exp via ACT, per-partition bias 0/-30000 folds the valid_lens mask into
    the softmax; 1/sqrt(D) folded into the ACT scale. P^T in fp16.
  - P @ [V | 1]: P^T slices stationary; column 64 of the accumulator is the
    softmax denominator. normalize = reciprocal + tensor_scalar_mul -> fp16
    out, upcast to fp32 on host.
Host does layout prep only (transpose/cast/pack/shard) - all FLOPs on device.
"""

import contextlib

import numpy as np

import concourse.bass as bass  # noqa: F401
import concourse.bacc as bacc
import concourse.mybir as mybir
import concourse.tile as tile
from concourse.bass_utils import run_bass_kernel_spmd

B, S, D = 64, 1024, 64
NCORES = 8
BPC = B // NCORES          # batches per core
NCH = S // 128             # k chunks of 128
NQT = S // 128             # q tiles of 128
F16 = mybir.dt.float16
F32 = mybir.dt.float32

# fused input row layout (f16 elements per partition):
#   [0:512)                q half (strip 0: q 0-511, strip 1: q 512-1023)
#   [512 : 512+128n)       K^T chunk c at 512+128c (same data in both strips)
#   [+ : +66n)             V' chunk c at +66c (65 used + 1 pad)
#   [+ : +n)               exp bias per chunk (0 / -30000, f16)
ROW = 512 + 195 * NCH  # 2072 (legacy single-row length, kept for bench scripts)
# split input rows: head tile unblocks QK+exp after ~1.3KB/partition;
# bulk tile (kt chunks 1.. + V') only gates later chunks and PV
AROW = 512 + NCH + 128        # [qt | bias(NCH) | kt chunk0]
BROW = 128 * (NCH - 1) + 66 * NCH  # [kt chunks 1.. | vv]

# DVE-exp offload: 1-op fp16 Schraudolph on the otherwise-idle vector
# engine. int16 = round(s * 128*log2e + (15*1024 - C)); those bits read
# as f16 give exp(s/8) * g(frac), where g is the piecewise-linear-2^x
# sawtooth. C=59 centers log(g) at 0, so mixed ACT(exact)/DVE(approx)
# chunks in one softmax row stay mutually unbiased; residual wiggle is
# ~[-3.9%, +2.0%] on DVE-chunk weights. numpy sim of the full pipeline
# with a 50% split: rel_err 7.4e-3 (threshold 2e-2). Only chunks that
# are fully valid for every batch in the slot group are eligible (the
# int16 path cannot apply the per-partition mask bias).
DVE_EVERY = 2
_LOG2E = 1.4426950408889634
_S16_A = 0.125 * 1024.0 * _LOG2E     # fold the 1/sqrt(D) score scale
_S16_C = 59.0                         # zero-mean-log sawtooth centering
_S16_B = 15.0 * 1024.0 - _S16_C

# modeled per-op costs (ns) for greedy ACT/DVE load balancing. DVE ops
# pay an extra pipe-DRAIN ~ max(0, op-266ns) before the next DVE op can
# issue (fits v2/v3 wall measurements within 0.3us), so effective DVE
# cost ~ 2*op-266. ScalarE shows no such penalty.
ACT_EXP_NS = (172 + 1024) / 1.2        # exp chunk, PSUM src
ACT_EXP2_NS = (172 + 2048) / 1.2       # fused 2-chunk exp
DVE_EXP_NS = 2 * (120 + 1024) / 0.96 - 266
DVE_EXP2_NS = 2 * (120 + 2048) / 0.96 - 266
DVE_R_NS = (124 + 97) / 0.96           # tsa + reciprocal (sub-drain size)
DVE_NORM_NS = 2 * (120 + 256) / 0.96 - 266   # broadcast mul per half
ACT_NORM_NS = 4 * (172 + 64) / 1.2     # 4 scale-AP Copy ops per half

_NC_CACHE = {}


def _build_nc(loop_reps=None, slot_counts=(NCH,) * BPC, ablate=frozenset()):
    # slot_counts entries: n (chunks to compute) or (n, nz) where chunks
    # 0..nz-2 are fully valid for EVERY batch dealt into that slot (group min
    # need) and may share a fused zero-bias exp; the rest always read their
    # per-chunk bias vector.
    nc = bacc.Bacc(None, target_bir_lowering=False)
    inpa = nc.dram_tensor("inpa", [BPC, 128, AROW], F16, kind="ExternalInput")
    inpb = nc.dram_tensor("inpb", [BPC, 128, BROW], F16, kind="ExternalInput")
    ot = nc.dram_tensor("ot", [BPC, 128, NQT, D], F16, kind="ExternalOutput")

    with tile.TileContext(nc) as tc:
        with (
            tc.tile_pool(name="inpool", bufs=3) as inpool,
            tc.tile_pool(name="ppool", bufs=16) as ppool,
            tc.tile_pool(name="ppool2", bufs=8) as ppool2,
            tc.tile_pool(name="outpool", bufs=2) as outpool,
            tc.tile_pool(name="rpool", bufs=4) as rpool,
            tc.tile_pool(name="spool", bufs=1, space="PSUM") as spool,
            tc.tile_pool(name="spool2", bufs=1, space="PSUM") as spool2,
            tc.tile_pool(name="accpool", bufs=1, space="PSUM") as accpool,
            tc.For_i(0, loop_reps, 1) if loop_reps else contextlib.nullcontext(),
        ):
            def emit_pv_block(prev, t):
                # one q-tile's full accumulation chain for the previous slot
                b_p, n_p, tb_p, pms_p, acc0_p, acc1_p = prev
                vo_p = 128 * (n_p - 1)
                acc = acc0_p if t < 4 else acc1_p
                for c in range(n_p):
                    pm_t, off = pms_p[c]
                    nc.tensor.matmul(
                        acc[:, t % 4, :],
                        lhsT=pm_t[:, off + t * 128:off + (t + 1) * 128],
                        rhs=tb_p[:, vo_p + 66 * c: vo_p + 66 * c + 65],
                        start=(c == 0), stop=(c == n_p - 1),
                    )

            def emit_dve_exp(st, pm, width):
                I16 = mybir.dt.int16
                nc.vector.tensor_scalar(
                    out=pm.bitcast(I16)[:, :width], in0=st[:, :width],
                    scalar1=_S16_A, scalar2=_S16_B,
                    op0=mybir.AluOpType.mult, op1=mybir.AluOpType.add)

            def emit_finish_half(prev, half, osb):
                # normalize + store one 4-q-tile half of the previous slot
                b_p, n_p, tb_p, pms_p, acc0_p, acc1_p = prev
                acc = acc0_p if half == 0 else acc1_p
                r = rpool.tile([128, 4], F32, tag="r", name="r")
                nc.vector.tensor_scalar_add(
                    out=r, in0=acc[:, :, D], scalar1=1e-30
                )
                nc.vector.reciprocal(r, r)
                eng_t[1] += DVE_R_NS
                emit_norm_mul(acc, r[:, :, None], half, osb)
                nc.sync.dma_start(
                    out=ot.ap()[b_p][:, half * 4:(half + 1) * 4, :],
                    in_=osb[:, half * 4:(half + 1) * 4, :],
                )

            def emit_norm_mul(acc, r3, half, osb):
                # r3: [128, 4, 1] reciprocal AP for this half's 4 q-tiles.
                # Greedy: one broadcast tensor_mul on DVE vs 4 scale-AP
                # Copy activations on ACT, whichever engine is behind.
                if eng_t[0] + ACT_NORM_NS < eng_t[1] + DVE_NORM_NS:
                    for t4 in range(4):
                        nc.scalar.activation(
                            out=osb[:, half * 4 + t4, :],
                            in_=acc[:, t4, 0:D],
                            func=mybir.ActivationFunctionType.Copy,
                            scale=r3[:, t4, :],
                        )
                    eng_t[0] += ACT_NORM_NS
                else:
                    nc.vector.tensor_mul(
                        out=osb[:, half * 4:(half + 1) * 4, :],
                        in0=acc[:, :, 0:D],
                        in1=r3.to_broadcast((128, 4, D)),
                    )
                    eng_t[1] += DVE_NORM_NS

            def emit_finish(prev):
                # merged-r variant: one reciprocal for both halves
                b_p, n_p, tb_p, pms_p, acc0_p, acc1_p = prev
                osb = outpool.tile([128, NQT, D], F16, name="osb")
                r = rpool.tile([128, 8], F32, tag="r8", name="r8")
                nc.vector.tensor_scalar_add(
                    out=r[:, 0:4], in0=acc0_p[:, :, D], scalar1=1e-30
                )
                nc.vector.tensor_scalar_add(
                    out=r[:, 4:8], in0=acc1_p[:, :, D], scalar1=1e-30
                )
                nc.vector.reciprocal(r, r)
                eng_t[1] += 2 * DVE_R_NS
                emit_norm_mul(acc0_p, r[:, 0:4, None], 0, osb)
                emit_norm_mul(acc1_p, r[:, 4:8, None], 1, osb)
                for half in range(2):
                    nc.sync.dma_start(
                        out=ot.ap()[b_p][:, half * 4:(half + 1) * 4, :],
                        in_=osb[:, half * 4:(half + 1) * 4, :],
                    )

            # tiny dummy exp: pulls the one-time ~2.7us ACT table load to
            # t=0 so it overlaps the first input DMA instead of serializing
            # before the first real exp
            warm = rpool.tile([128, 1], F32, tag="warm", name="warm")
            nc.vector.memset(warm, 0.0)
            nc.scalar.activation(
                out=warm, in_=warm, func=mybir.ActivationFunctionType.Exp
            )

            prev = None
            # modeled busy-ns [ACT, DVE]; ACT starts 2.7us behind (one-time
            # activation-table load at t=0, hidden behind the input DMA only
            # partially)
            eng_t = [2700.0, 0.0]
            pair_toggle = [True]
            for b in range(BPC):
                sc = slot_counts[b]
                n, nz = sc if isinstance(sc, tuple) else (sc, sc)
                n = max(1, min(NCH, n))
                nz = max(1, min(n, nz))
                fz = nz - 1  # chunks 0..fz-1 are zero-bias for all batches
                ua = 512 + n + 128
                ub = 128 * (n - 1) + 66 * n

                ta = inpool.tile([128, AROW], F16, tag="ta", name="ta")
                nc.sync.dma_start(out=ta[:, :ua], in_=inpa.ap()[b][:, :ua])
                tb = inpool.tile([128, BROW], F16, tag="tb", name="tb")
                nc.sync.dma_start(out=tb[:, :ub], in_=inpb.ap()[b][:, :ub])
                qt = ta[:, 0:512]

                acc0 = accpool.tile([128, 4, D + 1], F32, tag="acc0")
                acc1 = accpool.tile([128, 4, D + 1], F32, tag="acc1")

                # Units: chunks grouped 2-per-exp on the 4-bank pair tile
                # (one fused exp amortizes the fixed per-instr cost; with
                # the mask folded into V' any chunk can pair). Pair/single
                # tiles strictly alternate (globally, via pair_toggle) so
                # QK(next) overlaps exp(current) within 6 PSUM banks for
                # scores (pair 4 + single 2; bufs=1 each).
                units = []
                c = 0
                while c < n:
                    if pair_toggle[0] and c + 1 < n:
                        units.append((True, [c, c + 1]))
                        c += 2
                    else:
                        units.append((False, [c]))
                        c += 1
                    pair_toggle[0] = not pair_toggle[0]

                # interleave this slot's QK+exp with the previous slot's PV so
                # the in-order PE queue never parks ACT behind a PV burst
                pms = [None] * n
                nu = len(units)
                m = max(nu, NQT if prev else 0)
                pv_done = 0
                for i in range(m):
                    if prev is not None and "pv" not in ablate:
                        pv_goal = min(NQT, (NQT * (i + 1) + m - 1) // m)
                        while pv_done < pv_goal:
                            emit_pv_block(prev, pv_done)
                            pv_done += 1
                    if i < nu:
                        is_pair, chunks = units[i]
                        width = 1024 * len(chunks)
                        if is_pair:
                            st = spool2.tile([128, 2 * S], F32, tag="st2",
                                             name="st2")
                        else:
                            st = spool.tile([128, S], F32, tag="st",
                                            name="st")
                        if "qk" not in ablate:
                            for idx, c in enumerate(chunks):
                                kt = (
                                    ta[:, 512 + n:512 + n + 128] if c == 0
                                    else tb[:, 128 * (c - 1):128 * c]
                                )
                                o = idx * 1024
                                nc.tensor.matmul(
                                    st[:, o:o + 512],
                                    lhsT=kt[0:64, :], rhs=qt[0:64, :],
                                    start=True, stop=True,
                                )
                                nc.tensor.matmul(
                                    st[:, o + 512:o + 1024],
                                    lhsT=kt[64:128, :], rhs=qt[64:128, :],
                                    start=True, stop=True,
                                )
                        if "exp" not in ablate:
                            if is_pair:
                                pm = ppool2.tile([128, 2 * S], F16,
                                                 tag="pm2", name="pm2")
                                act_ns, dve_ns = ACT_EXP2_NS, DVE_EXP2_NS
                            else:
                                pm = ppool.tile([128, S], F16, tag="pm",
                                                name="pm")
                                act_ns, dve_ns = ACT_EXP_NS, DVE_EXP_NS
                            # greedy ACT/DVE balance on modeled op costs
                            on_dve = (
                                DVE_EVERY
                                and eng_t[1] + dve_ns
                                <= eng_t[0] + act_ns
                            )
                            if on_dve:
                                emit_dve_exp(st, pm, width)
                                eng_t[1] += dve_ns
                            else:
                                nc.scalar.activation(
                                    out=pm[:, :width], in_=st[:, :width],
                                    func=mybir.ActivationFunctionType.Exp,
                                    scale=0.125,
                                )
                                eng_t[0] += act_ns
                            for idx, c in enumerate(chunks):
                                pms[c] = (pm, idx * 1024)
                if prev is not None and "pv" not in ablate:
                    while pv_done < NQT:
                        emit_pv_block(prev, pv_done)
                        pv_done += 1
                if prev is not None:
                    if "pv" not in ablate:
                        emit_finish(prev)
                    else:
                        b_p, n_p, tb_p, pms_p = prev[:4]
                        src = pms_p[-1][0] if "exp" not in ablate else tb_p
                        nc.sync.dma_start(
                            out=ot.ap()[b_p],
                            in_=src[:, 0:NQT * D].rearrange(
                                "p (t d) -> p t d", d=D
                            ),
                        )
                prev = (b, n, tb, pms, acc0, acc1)

            # drain the last slot: finish+store half 0 while half 1's PV runs
            if "pv" not in ablate:
                osb = outpool.tile([128, NQT, D], F16, name="osb")
                for t in range(NQT):
                    emit_pv_block(prev, t)
                    if t == 3:
                        emit_finish_half(prev, 0, osb)
                emit_finish_half(prev, 1, osb)
            else:
                b_p, n_p, tin_p, pms_p = prev[:4]
                src = pms_p[-1][0] if "exp" not in ablate else tin_p
                nc.sync.dma_start(
                    out=ot.ap()[b_p],
                    in_=src[:, 0:NQT * D].rearrange("p (t d) -> p t d", d=D),
                )

    nc.compile()
    return nc


def _get_nc(slot_counts=(NCH,) * BPC):
    key = tuple(slot_counts)
    if key not in _NC_CACHE:
        _NC_CACHE[key] = _build_nc(slot_counts=key)
    return _NC_CACHE[key]


def _host_prep(queries, keys, values, valid_lens):
    queries = np.asarray(queries, dtype=np.float32)
    keys = np.asarray(keys, dtype=np.float32)
    values = np.asarray(values, dtype=np.float32)
    lens = np.asarray(valid_lens).astype(np.int64)

    q16 = queries.astype(np.float16)
    k16 = keys.astype(np.float16)
    v16 = values.astype(np.float16)

    # q halves packed into the two PE row strips: [B, 128, 512]
    qh = q16.transpose(0, 2, 1).reshape(B, 64, 2, 512)
    qh = np.ascontiguousarray(qh.transpose(0, 2, 1, 3)).reshape(B, 128, 512)

    # K^T chunks duplicated into both strips: [B, 128, NCH, 128]
    kt4 = k16.transpose(0, 2, 1).reshape(B, 64, NCH, 128)
    ktd = np.concatenate([kt4, kt4], axis=1)

    # V with ones column (pad to 66): [B, 128, NCH, 66]. The valid_lens
    # mask is folded HERE: masked k rows get V=0 AND ones=0, so their P
    # values (exp of real scores, always finite) contribute nothing to
    # numerator or denominator. No exp bias needed -> every chunk can go
    # to either exp engine.
    kpos = np.arange(S).reshape(NCH, 128).T  # [128, NCH] -> k = c*128 + p
    vmask = kpos[None] < lens[:, None, None]  # [B, 128, NCH]
    vp = np.zeros((B, 128, NCH, D + 2), np.float16)
    vp[:, :, :, :D] = (
        v16.reshape(B, NCH, 128, D).transpose(0, 2, 1, 3)
        * vmask[..., None]
    )
    vp[:, :, :, D] = vmask.astype(np.float16)

    # legacy bias lanes (kept in the layout, no longer read on device)
    bia = np.zeros((B, 128, NCH), np.float16)

    # Length specialization: batch i needs ceil(L_i/128) k-chunks (min 1).
    # Sort by need, deal round-robin -> every core's slot s holds batches of
    # (near-)equal need; slot count = max within the deal group, so all cores
    # run the identical compiled program, perfectly balanced.
    need = np.maximum(1, -(-lens // 128)).astype(np.int64)
    order = np.argsort(need, kind="stable")
    gmax = [int(need[order[g * NCORES:(g + 1) * NCORES]].max()) for g in range(BPC)]
    gmin = [int(need[order[g * NCORES:(g + 1) * NCORES]].min()) for g in range(BPC)]
    # second-smallest slot FIRST (short pipeline fill: PV can't start until
    # slot 0's exps all land), then descending, smallest LAST (tiny drain)
    perm = [1] + list(range(BPC - 1, 1, -1)) + [0]
    slot_counts = tuple((gmax[p], gmin[p]) for p in perm)

    in_maps = []
    for c in range(NCORES):
        fa = np.zeros((BPC, 128, AROW), np.float16)
        fb = np.zeros((BPC, 128, BROW), np.float16)
        for s in range(BPC):
            n = slot_counts[s][0]
            b = int(order[perm[s] * NCORES + c])
            fa[s, :, 0:512] = qh[b]
            fa[s, :, 512:512 + n] = bia[b, :, :n]
            fa[s, :, 512 + n:512 + n + 128] = ktd[b, :, 0]
            if n > 1:
                fb[s, :, :128 * (n - 1)] = (
                    ktd[b, :, 1:n].reshape(128, 128 * (n - 1))
                )
            vo = 128 * (n - 1)
            fb[s, :, vo:vo + 66 * n] = vp[b, :, :n, :66].reshape(128, 66 * n)
        in_maps.append({"inpa": fa, "inpb": fb})
    return slot_counts, order, perm, in_maps


def kernel(queries, keys, values, valid_lens):
    slot_counts, order, perm, in_maps = _host_prep(
        queries, keys, values, valid_lens
    )
    nc = _get_nc(slot_counts)
    res = run_bass_kernel_spmd(nc, in_maps, core_ids=list(range(NCORES)))

    out = np.empty((B, S, D), np.float32)
    for c in range(NCORES):
        otv = res.results[c]["ot"]  # [BPC, 128, NQT, D] f16
        ids = [int(order[perm[s] * NCORES + c]) for s in range(BPC)]
        out[ids] = otv.transpose(0, 2, 1, 3).reshape(BPC, S, D).astype(np.float32)
    return out

